# revision 21
# baseline (speedup 1.0000x reference)
"""Trainium2 Bass kernel for nn_CausalSE: causal cumulative-average pooling
+ squeeze-excite gating, data-parallel over batch (one NeuronCore per batch
element).

Reference math per batch element (D=512, T=8192, chunk=16, Tc=512):
    avg    = cumsum(x, t) / (t+1)
    pooled = avg[:, 15::16]                          # [D, Tc]
    h      = relu(w1 @ pooled + b1)                  # [64, Tc]
    g      = sigmoid(w2 @ h + b2)                    # [D, Tc]
    out    = repeat(g, 16, t)[:, :T] * x

The kernel is HBM-bound: per core it streams x in and out once.  x
crosses HBM as fp16 (host converts both ways): ~16.8 MB at the ~400
GB/s per-core aggregate DMA rate => ~42us floor + ~10us of fixed NEFF
preamble/postamble, so every compute engine must stay under ~40us and
the per-block serial chain (load -> w1-matmul -> scan -> gate-matmul
-> sigmoid -> multiply -> store) must pipeline across blocks.

Structure (v1 was DVE-bound at ~55us busy; failed experiments: PSUM
same-address broadcast-accumulate output APs lose updates (RMW
hazard), strided moving-operand matmuls run ~5x slow):
  - Chunk sums ride the (otherwise idle) PE: w1 @ chunk_sum(x) ==
    chunk_sum(w1 @ x), so the PE computes Y = w1 @ x (4 ki
    accumulation steps into PSUM [64, 512] banks) and DVE
    windowed-reduces the 8x-smaller Y.
  - relu rides DVE as tensor_scalar(add b1, max 0); b2 is folded into
    the gate matmul (h gets a constant 1.0 row 64, w2 a b2 row), so
    the 4 per-block sigmoid+16x-upsample ACTIVATEs merge into ONE
    (saves the 352-cycle ACT fixed cost 18x and all ACT bias reads)
    and ACT only ever needs the sigmoid table set (a dummy 1-element
    sigmoid up front pulls the table load into the startup window).
  - Two-level software pipelining: the gate matmuls + sigmoid of
    block k are emitted after block k+1's Y-matmuls (PE runs the
    Y-stream dense, keeping the HAM clock-gate warm), and the gate
    multiplies + stores of block k are emitted two iterations behind
    (DVE never waits on ACT).
  - DMA: one load per t-block carrying all 4 d-tiles; block 0 is
    small (256 cols) and goes FIRST on the SP ring so compute ramps
    at ~10us; w1 is host-pre-swizzled partition-major (the naive
    (d p)->p d rearrange makes 128-byte descriptors that crawl); b1
    rides column 0 of the scale tensor.  Stores pair d-tiles: d01 on
    the SP ring, d23 on the ACT ring.  GpSimd issues nothing.
"""

import sys

for _p in ("/opt/trn_rl_repo",):
    if _p not in sys.path:
        sys.path.insert(0, _p)

import numpy as np

B, D, T = 8, 512, 8192
DH = 64          # bottleneck dim = D // 8
CS = 16          # chunksize
TC = T // CS     # 512 chunks
NCORES = 8
NDT = D // 128   # 4 partition tiles of x / out
SB = 512         # max Y-matmul sub-block (one PSUM bank of fp32)
TBLOCKS = [(0, 256), (256, 512), (768, 1024), (1792, 1536), (3328, 2048),
           (5376, 1792), (7168, 768), (7936, 256)]
TBMAX = 2048

_compiled_nc = None


def build_nc():
    import concourse.tile as tile
    from concourse import bacc, mybir

    f32 = mybir.dt.float32
    f16 = mybir.dt.float16
    AF = mybir.ActivationFunctionType
    ALU = mybir.AluOpType
    AX = mybir.AxisListType

    # Bacc (not plain Bass): its finalize() runs the TRN2 sync-wait
    # legalization (move_matmul_waits_to_ldweights / event-semaphore
    # splitting) that walrus codegen requires.
    nc = bacc.Bacc("TRN2", target_bir_lowering=False)
    x_d = nc.declare_dram_parameter("x", [D, T], f16, isOutput=False)
    w1p_d = nc.declare_dram_parameter("w1p", [128, NDT * DH], f16,
                                      isOutput=False)
    w2e_d = nc.declare_dram_parameter("w2e", [DH + 1, D], f16, isOutput=False)
    sclb_d = nc.declare_dram_parameter("sclb", [DH, TC + 1], f32,
                                       isOutput=False)
    out_d = nc.declare_dram_parameter("out", [D, T], f16, isOutput=True)

    with tile.TileContext(nc) as tc:
        with (
            tc.tile_pool(name="xres", bufs=1) as xres,
            tc.tile_pool(name="small", bufs=1) as small,
            tc.tile_pool(name="ups", bufs=3) as ups,
            tc.tile_pool(name="psum_y", bufs=1, space="PSUM") as psum_y,
            tc.tile_pool(name="psum_g", bufs=2, space="PSUM") as psum_g,
        ):
            # x resident in SBUF: [128, 4, 8192] fp16 = 8 MB
            xt = xres.tile([128, NDT, T], f16, tag="x", name="x")
            w1s = small.tile([128, NDT, DH], f16, tag="w1")
            w2s = small.tile([DH + 1, D], f16, tag="w2")
            sclb = small.tile([DH, TC + 1], f32, tag="sclb")
            b1s = sclb[:, 0:1]
            scl = sclb[:, 1:TC + 1]
            q = small.tile([DH, TC], f32, tag="q")      # per-chunk w1@x sums
            qs = small.tile([DH, TC], f32, tag="qs")    # causal prefix
            h32 = small.tile([DH, TC], f32, tag="h32")
            # h with a constant 1.0 row DH that turns the gate matmul's
            # extra w2-row (= b2) into the bias add
            h16 = small.tile([DH + 1, TC], f16, tag="h16")
            yp = [
                psum_y.tile([DH, SB], f32, tag=f"y{sb}", name=f"y{sb}")
                for sb in range(4)
            ]

            nc.vector.memset(h16[DH:DH + 1, :], 1.0)

            # Dummy 1-element sigmoid: forces the walrus-inserted
            # ACT_TABLE_LOAD for the sigmoid set to run during the startup
            # DMA window instead of stalling ACT before the first real
            # sigmoid mid-stream.
            dummy = small.tile([1, 2], f32, tag="dummy")
            nc.gpsimd.memset(dummy[:], 0.0)
            nc.scalar.activation(dummy[:, 1:2], dummy[:, 0:1], AF.Sigmoid)

            def load_block(eng, t0, TB):
                eng.dma_start(
                    xt[:, :, t0:t0 + TB],
                    x_d[:, t0:t0 + TB].rearrange("(k p) t -> p k t", p=128),
                )

            # Loads alternate rings block-by-block so completion order
            # matches the compute's need order (consecutive blocks ride
            # different rings and overlap).  SP ring: block 0 first, then
            # the weights its chain needs; d01 stores follow later.
            load_block(nc.sync, *TBLOCKS[0])
            nc.sync.dma_start(
                w1s[:], w1p_d[:].rearrange("p (d h) -> p d h", d=NDT)
            )
            nc.sync.dma_start(sclb[:], sclb_d[:])
            for bi in (2, 4, 6):
                load_block(nc.sync, *TBLOCKS[bi])
            # ACT ring: the gate weights + odd loads, all issued up front
            # before the first sigmoid; d23 stores follow later.
            nc.scalar.dma_start(w2s[:], w2e_d[:])
            for bi in (1, 3, 5, 7):
                load_block(nc.scalar, *TBLOCKS[bi])

            sbg = 0  # rotating PSUM bank assignment for Y sub-blocks

            def prefix_stage(tb):
                """Y-matmuls + reduces + scan + scale + bias-relu."""
                nonlocal sbg
                t0, TB = TBLOCKS[tb]
                CB = TB // CS
                c0 = t0 // CS
                subs = [SB] * (TB // SB) + ([TB % SB] if TB % SB else [])
                banks = [(sbg + i) % 4 for i in range(len(subs))]
                sbg += len(subs)
                for ki in range(NDT):
                    ts = t0
                    for sb, w in enumerate(subs):
                        nc.tensor.matmul(
                            yp[banks[sb]][:, :w],
                            w1s[:, ki, :],
                            xt[:, ki, ts:ts + w],
                            start=(ki == 0),
                            stop=(ki == NDT - 1),
                        )
                        ts += w
                cc = c0
                for sb, w in enumerate(subs):
                    nc.vector.reduce_sum(
                        q[:, cc:cc + w // CS],
                        yp[banks[sb]][:, :w].rearrange(
                            "p (c j) -> p c j", j=CS),
                        axis=AX.X,
                    )
                    cc += w // CS
                nc.vector.tensor_tensor_scan(
                    qs[:, c0:c0 + CB],
                    q[:, c0:c0 + CB],
                    q[:, c0:c0 + CB],
                    0.0 if tb == 0 else qs[:, c0 - 1:c0],
                    op0=ALU.add,
                    op1=ALU.bypass,
                )
                nc.vector.tensor_mul(
                    h32[:, c0:c0 + CB], qs[:, c0:c0 + CB], scl[:, c0:c0 + CB]
                )
                nc.vector.tensor_scalar(
                    h16[:DH, c0:c0 + CB], h32[:, c0:c0 + CB],
                    b1s, 0.0, op0=ALU.add, op1=ALU.max,
                )

            def gate_stage(tb):
                """4 gate matmuls (bias via the 1.0 h-row) + ONE merged
                sigmoid + 16x upsample ACTIVATE for all 4 d-tiles."""
                t0, TB = TBLOCKS[tb]
                CB = TB // CS
                c0 = t0 // CS
                gp = psum_g.tile([128, NDT, TBMAX // CS], f32, tag="g",
                                 name="gp")
                for di in range(NDT):
                    nc.tensor.matmul(
                        gp[:, di, :CB],
                        w2s[:, di * 128:(di + 1) * 128],
                        h16[:, c0:c0 + CB],
                        start=True,
                        stop=True,
                    )
                u = ups.tile([128, NDT, TBMAX], f16, tag="u", name="u")
                nc.scalar.activation(
                    u[:, :, :TB].rearrange("p k (c j) -> p k c j", j=CS),
                    gp[:, :, :CB].unsqueeze(3).broadcast_to(
                        [128, NDT, CB, CS]),
                    AF.Sigmoid,
                )
                return u

            def mult_stage(tb, u, tail=False):
                """Gate multiplies + stores (d01 -> SP ring, d23 -> ACT)."""
                t0, TB = TBLOCKS[tb]
                for di in range(NDT):
                    xv = xt[:, di, t0:t0 + TB]
                    nc.vector.tensor_tensor(
                        xv, xv, u[:, di, :TB], op=ALU.mult
                    )
                    if tail:
                        deng = nc.sync if di < 2 else nc.scalar
                        deng.dma_start(
                            out_d[di * 128:(di + 1) * 128, t0:t0 + TB], xv
                        )
                    elif di % 2 == 1:
                        half = di // 2
                        deng = nc.sync if half == 0 else nc.scalar
                        deng.dma_start(
                            out_d[half * 256:(half + 1) * 256,
                                  t0:t0 + TB].rearrange(
                                      "(k p) t -> p k t", p=128),
                            xt[:, 2 * half:2 * half + 2, t0:t0 + TB],
                        )

            NB = len(TBLOCKS)
            ulist = {}
            for tb in range(NB):
                prefix_stage(tb)
                if tb >= 1:
                    ulist[tb - 1] = gate_stage(tb - 1)
                if tb >= 2:
                    mult_stage(tb - 2, ulist.pop(tb - 2))
            ulist[NB - 1] = gate_stage(NB - 1)
            mult_stage(NB - 2, ulist.pop(NB - 2))
            mult_stage(NB - 1, ulist.pop(NB - 1), tail=True)
    # run_bass_via_pjrt serializes nc.m as-is; Bacc defers register
    # allocation and TRN2 sync-wait legalization to finalize(), so it must
    # run here or walrus rejects the BIR.
    nc.finalize()
    return nc


def _host_inputs(x, w1, b1, w2, b2, chunksize):
    x = np.asarray(x)
    w1 = np.asarray(w1, dtype=np.float32)
    b1 = np.ascontiguousarray(np.asarray(b1, dtype=np.float32))
    w2 = np.asarray(w2, dtype=np.float32)
    b2 = np.asarray(b2, dtype=np.float32)
    cs = int(chunksize)
    assert cs == CS and x.shape == (B, D, T), (cs, x.shape)
    x16 = np.ascontiguousarray(x.astype(np.float16))
    # w1 pre-swizzled partition-major: w1p[p, k*DH+h] = w1[h, k*128+p]
    w1p = np.ascontiguousarray(
        w1.T.astype(np.float16).reshape(NDT, 128, DH)
        .transpose(1, 0, 2).reshape(128, NDT * DH)
    )
    # w2 transposed with b2 as the extra row DH (paired with h's 1.0 row)
    w2e = np.ascontiguousarray(np.concatenate(
        [w2.T, b2[None, :]], axis=0).astype(np.float16))     # [DH+1, D]
    scale = 1.0 / (CS * np.arange(1, TC + 1, dtype=np.float32))
    sclb = np.ascontiguousarray(np.concatenate(
        [np.broadcast_to(b1[:, None], (DH, 1)),
         np.broadcast_to(scale, (DH, TC))], axis=1,
    ))
    shared = dict(w1p=w1p, w2e=w2e, sclb=sclb)
    return x16, shared


def kernel(x, w1, b1, w2, b2, chunksize):
    global _compiled_nc
    from concourse.bass_utils import run_bass_kernel_spmd

    x16, shared = _host_inputs(x, w1, b1, w2, b2, chunksize)
    if _compiled_nc is None:
        _compiled_nc = build_nc()
    in_maps = [
        {"x": np.ascontiguousarray(x16[i]), **shared} for i in range(NCORES)
    ]
    res = run_bass_kernel_spmd(_compiled_nc, in_maps, list(range(NCORES)))
    out = np.stack(
        [res.results[i]["out"] for i in range(NCORES)], axis=0
    ).astype(np.float32)
    return out
